# revision 9
# baseline (speedup 1.0000x reference)
"""CorrelationLayer1D Trainium2 Bass kernel (v3: bf16 + software pipelining).

Computes out[b, d, h, w] = sum_c x_1[b,c,h,w] * x2p[b,c,h,w+d] for d in [0, 41),
where x2p is x_2 width-padded by (8, 32).  Inputs [4,128,160,320] f32.

Sharding: data-parallel over H = 160 = 8*20 (correlation runs along W only, so
H-sharding needs no halo).  Per core, rows are processed in chunks of HC=10.

  Loads are fp32->bf16 casting DMAs (SWDGE/gpsimd, the only engine that casts
  in-flight); x_2 is loaded UNPADDED and the (8,32) zero-pad is realised by
  narrowing the two edge matmul windows and memsetting the corresponding
  atlas corners, so every load packet is a full 12.8KB partition row.

  per (b, h) row, per w-block (128/128/64):
    PE computes a block-diagonal Gram in bf16: two M=64 matmuls land as one
    compacted [128,104] PSUM f32 tile - the 41-wide correlation band of
    partition i lives at free offset (i mod 64)+d.  Matmuls for 4 rows sit
    side by side in one [128, 4*104] PSUM tile; per (wblock, half) one
    scalar/vector copy casts each [64, 4*104] half into its own bf16 atlas
    tile so the scratch store for that slice can launch immediately.
  Per chunk, 5 DMAs write atlas slices to DRAM scratch and 5 skewed reloads
  (flat DRAM-side APs, step hc*GW+1) extract the band as S[i, (h,d)] - DRAM
  APs allow arbitrary affine steps; SBUF-side per-partition skews are not
  (neuronx-cc "illegal partition step"), which is why the round-trip exists.
  The transpose/assembly/output stage of chunk N is emitted AFTER chunk
  N+1's grams, so the PE never stalls on the scratch round-trip latency:
  per row, 3 PE transposes gather S into a [41, 320] PSUM tile and one
  vector/scalar copy casts it into the assembly tile; one DMA per chunk
  stores [41, hc, 320] to DRAM.
"""

import sys

import numpy as np

try:
    import concourse.bass as bass  # noqa: F401
except ImportError:
    sys.path.insert(0, "/opt/trn_rl_repo")

import concourse.bass as bass
import concourse.tile as tile
from concourse import bacc, masks, mybir
from concourse.ap import AP
from concourse.bass_utils import run_bass_kernel_spmd

MAX_DISP = 40
D = MAX_DISP + 1  # 41 displacements
PAD_L = 8
PAD_R = 32
B, C, H, W = 4, 128, 160, 320
N_CORES = 8
HS = H // N_CORES  # 20 h-rows per core
WBLOCKS = [(0, 128), (128, 128), (256, 64)]
GW = 104  # compacted gram width per block: 64 + MAX_DISP
SLICES = [(0, 0), (0, 1), (1, 0), (1, 1), (2, 0)]  # (wblock, half)

F32 = mybir.dt.float32
BF16 = mybir.dt.bfloat16


def build_kernel(b_dim=B, hs=HS, hc=None):
    if hc is None:
        hc = 10 if hs % 10 == 0 else hs
    assert hs % hc == 0
    nchunks = hs // hc

    nc = bacc.Bacc("TRN2", target_bir_lowering=False, debug=False)
    x1e = nc.declare_dram_parameter("x1", [b_dim, C, hs, W], F32, isOutput=False)
    x2e = nc.declare_dram_parameter("x2", [b_dim, C, hs, W], F32, isOutput=False)
    oute = nc.declare_dram_parameter("out", [b_dim, D, hs, W], F32, isOutput=True)

    with tile.TileContext(nc) as tc:
        with (
            tc.tile_pool(name="const", bufs=1) as const_pool,
            tc.tile_pool(name="xin", bufs=3) as xin_pool,
            tc.tile_pool(name="atlas", bufs=2) as atlas_pool,
            tc.tile_pool(name="sbig", bufs=2) as sbig_pool,
            tc.tile_pool(name="asm", bufs=2) as asm_pool,
            tc.tile_pool(name="psum_g", bufs=2, space="PSUM") as psum_g,
            tc.tile_pool(name="psum_t", bufs=2, space="PSUM") as psum_t,
            tc.tile_pool(name="scratch", bufs=2, space="DRAM") as scratch_pool,
        ):
            identity = const_pool.tile([128, 128], BF16)
            masks.make_identity(nc, identity[:])

            def emit_front(b, ci):
                """Loads, grams, atlas copies, scratch store+reload."""
                h0 = ci * hc
                x1b = xin_pool.tile(
                    [C, hc * W], BF16, tag="x1b", name=f"x1b_{b}_{ci}"
                )
                nc.gpsimd.dma_start(
                    x1b[:].rearrange("p (h w) -> p h w", w=W),
                    x1e[b, :, h0 : h0 + hc, :],
                )
                x2b = xin_pool.tile(
                    [C, hc * W], BF16, tag="x2b", name=f"x2b_{b}_{ci}"
                )
                nc.gpsimd.dma_start(
                    x2b[:].rearrange("p (h w) -> p h w", w=W),
                    x2e[b, :, h0 : h0 + hc, :],
                )

                atlases = {
                    (k, hf): atlas_pool.tile(
                        [64, hc * GW],
                        BF16,
                        tag=f"at{k}{hf}",
                        name=f"at{k}{hf}_{b}_{ci}",
                    )
                    for k, hf in SLICES
                }

                ncop = 0
                for g0 in range(0, hc, 4):
                    ng = min(4, hc - g0)
                    ps = [
                        psum_g.tile(
                            [wb, ng * GW],
                            F32,
                            tag=f"g{k}",
                            name=f"g{k}_{b}_{ci}_{g0}",
                        )
                        for k, (w0, wb) in enumerate(WBLOCKS)
                    ]
                    for q in range(ng):
                        hh = g0 + q
                        o1 = hh * W
                        o2 = hh * W
                        for kblk, (w0, wb) in enumerate(WBLOCKS):
                            for half in range(wb // 64):
                                ws = w0 + 64 * half  # global column of row 0
                                # window in unpadded x2 coords: [ws-8, ws+96)
                                lo = max(ws - PAD_L, 0)
                                hi = min(ws + GW - PAD_L, W)
                                joff = lo - (ws - PAD_L)  # left clip
                                nc.tensor.matmul(
                                    ps[kblk][
                                        half * 64 : half * 64 + 64,
                                        q * GW + joff : q * GW + joff + (hi - lo),
                                    ],
                                    x1b[:, o1 + ws : o1 + ws + 64],
                                    x2b[:, o2 + lo : o2 + hi],
                                    start=True,
                                    stop=True,
                                    tile_position=(0, 64 * half),
                                )
                    for kblk, hf in SLICES:
                        src = ps[kblk][hf * 64 : (hf + 1) * 64, :]
                        dst = atlases[(kblk, hf)][:, g0 * GW : (g0 + ng) * GW]
                        if ncop % 2 == 0:
                            nc.scalar.copy(dst, src)
                        else:
                            nc.vector.tensor_copy(dst, src)
                        ncop += 1

                # zero the pad corners the narrowed edge matmuls skipped:
                #  (i<8, j<8) of slice (0,0) and (i>=32, j>=72) of slice (2,0)
                a00 = atlases[(0, 0)][:].rearrange("p (h g) -> p h g", g=GW)
                nc.gpsimd.memset(a00[0:PAD_L, :, 0:PAD_L], 0.0)
                a20 = atlases[(2, 0)][:].rearrange("p (h g) -> p h g", g=GW)
                nc.gpsimd.memset(a20[PAD_R : 64, :, GW - PAD_R : GW], 0.0)

                sbig = [
                    sbig_pool.tile(
                        [128, hc * D], BF16, tag="sbig0", name=f"sbig0_{b}_{ci}"
                    ),
                    sbig_pool.tile(
                        [128, hc * D], BF16, tag="sbig1", name=f"sbig1_{b}_{ci}"
                    ),
                    sbig_pool.tile(
                        [64, hc * D], BF16, tag="sbig2", name=f"sbig2_{b}_{ci}"
                    ),
                ]
                for si, (kblk, hf) in enumerate(SLICES):
                    scr = scratch_pool.tile(
                        [64, hc * GW],
                        BF16,
                        tag=f"scr{kblk}{hf}",
                        name=f"scr{kblk}{hf}_{b}_{ci}",
                    )
                    store_eng = nc.sync if si % 2 == 0 else nc.scalar
                    store_eng.dma_start(scr[:], atlases[(kblk, hf)][:])

                    scr_ap = scr[:]
                    diag = AP(
                        tensor=scr_ap.tensor,
                        offset=scr_ap.offset,
                        ap=[[hc * GW + 1, 64], [GW, hc], [1, D]],
                    )
                    dstp = sbig[kblk][hf * 64 : (hf + 1) * 64, :].rearrange(
                        "p (h d) -> p h d", d=D
                    )
                    load_eng = nc.scalar if si % 2 == 0 else nc.sync
                    load_eng.dma_start(dstp, diag)
                return sbig

            def emit_back(b, ci, sbig):
                """Transposes, assembly copies, output store."""
                h0 = ci * hc
                abatch = asm_pool.tile(
                    [D, hc * W], F32, tag="abatch", name=f"abatch_{b}_{ci}"
                )
                for hh in range(hc):
                    t_ps = psum_t.tile(
                        [D, W], BF16, tag="t_ps", name=f"t_ps_{b}_{ci}_{hh}"
                    )
                    for kblk, (w0, wb) in enumerate(WBLOCKS):
                        nc.tensor.matmul(
                            t_ps[:, w0 : w0 + wb],
                            sbig[kblk][0:wb, hh * D : (hh + 1) * D],
                            identity[0:wb, 0:wb],
                            start=True,
                            stop=True,
                            is_transpose=True,
                        )
                    dst = abatch[:, hh * W : (hh + 1) * W]
                    if hh % 2 == 0:
                        nc.vector.tensor_copy(dst, t_ps[:])
                    else:
                        nc.scalar.copy(dst, t_ps[:])

                eng = nc.sync if ci % 2 == 0 else nc.scalar
                eng.dma_start(
                    oute[b, :, h0 : h0 + hc, :],
                    abatch[:].rearrange("d (h w) -> d h w", w=W),
                )

            prev = None
            for b in range(b_dim):
                for ci in range(nchunks):
                    sbig = emit_front(b, ci)
                    if prev is not None:
                        emit_back(*prev)
                    prev = (b, ci, sbig)
            emit_back(*prev)

    nc.finalize()
    return nc


_compiled = {}


def _get_kernel(b_dim, hs):
    key = (b_dim, hs)
    if key not in _compiled:
        _compiled[key] = build_kernel(b_dim, hs)
    return _compiled[key]


def kernel(x_1: np.ndarray, x_2: np.ndarray) -> np.ndarray:
    assert x_1.shape == (B, C, H, W) and x_2.shape == (B, C, H, W)
    x_1 = np.ascontiguousarray(x_1, dtype=np.float32)
    x_2 = np.ascontiguousarray(x_2, dtype=np.float32)
    nc = _get_kernel(B, HS)
    in_maps = [
        {
            "x1": np.ascontiguousarray(x_1[:, :, i * HS : (i + 1) * HS, :]),
            "x2": np.ascontiguousarray(x_2[:, :, i * HS : (i + 1) * HS, :]),
        }
        for i in range(N_CORES)
    ]
    res = run_bass_kernel_spmd(nc, in_maps, core_ids=list(range(N_CORES))).results
    out = np.concatenate([res[i]["out"] for i in range(N_CORES)], axis=2)
    return out


# revision 11
# speedup vs baseline: 1.0683x; 1.0683x over previous
"""CorrelationLayer1D Trainium2 Bass kernel (v3: bf16 + software pipelining).

Computes out[b, d, h, w] = sum_c x_1[b,c,h,w] * x2p[b,c,h,w+d] for d in [0, 41),
where x2p is x_2 width-padded by (8, 32).  Inputs [4,128,160,320] f32.

Sharding: data-parallel over H = 160 = 8*20 (correlation runs along W only, so
H-sharding needs no halo).  Per core, rows are processed in chunks of HC=10.

  Loads are fp32->bf16 casting DMAs (SWDGE/gpsimd, the only engine that casts
  in-flight); x_2 is loaded UNPADDED and the (8,32) zero-pad is realised by
  narrowing the two edge matmul windows and memsetting the corresponding
  atlas corners, so every load packet is a full 12.8KB partition row.

  per (b, h) row, per w-block (128/128/64):
    PE computes a block-diagonal Gram in bf16: two M=64 matmuls land as one
    compacted [128,104] PSUM f32 tile - the 41-wide correlation band of
    partition i lives at free offset (i mod 64)+d.  Matmuls for 4 rows sit
    side by side in one [128, 4*104] PSUM tile; per (wblock, half) one
    scalar/vector copy casts each [64, 4*104] half into its own bf16 atlas
    tile so the scratch store for that slice can launch immediately.
  Per chunk, 5 DMAs write atlas slices to DRAM scratch and 5 skewed reloads
  (flat DRAM-side APs, step hc*GW+1) extract the band as S[i, (h,d)] - DRAM
  APs allow arbitrary affine steps; SBUF-side per-partition skews are not
  (neuronx-cc "illegal partition step"), which is why the round-trip exists.
  The transpose/assembly/output stage of chunk N is emitted AFTER chunk
  N+1's grams, so the PE never stalls on the scratch round-trip latency:
  per row, 3 PE transposes gather S into a [41, 320] PSUM tile and one
  vector/scalar copy casts it into the assembly tile; one DMA per chunk
  stores [41, hc, 320] to DRAM.
"""

import sys

import numpy as np

try:
    import concourse.bass as bass  # noqa: F401
except ImportError:
    sys.path.insert(0, "/opt/trn_rl_repo")

import concourse.bass as bass
import concourse.tile as tile
from concourse import bacc, masks, mybir
from concourse.ap import AP
from concourse.bass_utils import run_bass_kernel_spmd

MAX_DISP = 40
D = MAX_DISP + 1  # 41 displacements
PAD_L = 8
PAD_R = 32
B, C, H, W = 4, 128, 160, 320
N_CORES = 8
HS = H // N_CORES  # 20 h-rows per core
WBLOCKS = [(0, 128), (128, 128), (256, 64)]
GW = 104  # compacted gram width per block: 64 + MAX_DISP
SLICES = [(0, 0), (0, 1), (1, 0), (1, 1), (2, 0)]  # (wblock, half)

F32 = mybir.dt.float32
BF16 = mybir.dt.bfloat16


def build_kernel(b_dim=B, hs=HS, hc=None):
    if hc is None:
        hc = 20 if hs % 20 == 0 else hs
    assert hs % hc == 0
    nchunks = hs // hc

    nc = bacc.Bacc("TRN2", target_bir_lowering=False, debug=False)
    x1e = nc.declare_dram_parameter("x1", [b_dim, C, hs, W], F32, isOutput=False)
    x2e = nc.declare_dram_parameter("x2", [b_dim, C, hs, W], F32, isOutput=False)
    oute = nc.declare_dram_parameter("out", [b_dim, D, hs, W], F32, isOutput=True)

    with tile.TileContext(nc) as tc:
        with (
            tc.tile_pool(name="const", bufs=1) as const_pool,
            tc.tile_pool(name="xin", bufs=3) as xin_pool,
            tc.tile_pool(name="atlas", bufs=2) as atlas_pool,
            tc.tile_pool(name="sbig", bufs=2) as sbig_pool,
            tc.tile_pool(name="asm", bufs=2) as asm_pool,
            tc.tile_pool(name="psum_g", bufs=2, space="PSUM") as psum_g,
            tc.tile_pool(name="psum_t", bufs=2, space="PSUM") as psum_t,
            tc.tile_pool(name="scratch", bufs=2, space="DRAM") as scratch_pool,
        ):
            identity = const_pool.tile([128, 128], BF16)
            masks.make_identity(nc, identity[:])

            def emit_front(b, ci):
                """Loads, grams, atlas copies, scratch store+reload."""
                h0 = ci * hc
                x1b = xin_pool.tile(
                    [C, hc * W], BF16, tag="x1b", name=f"x1b_{b}_{ci}"
                )
                nc.gpsimd.dma_start(
                    x1b[:].rearrange("p (h w) -> p h w", w=W),
                    x1e[b, :, h0 : h0 + hc, :],
                )
                x2b = xin_pool.tile(
                    [C, hc * W], BF16, tag="x2b", name=f"x2b_{b}_{ci}"
                )
                nc.gpsimd.dma_start(
                    x2b[:].rearrange("p (h w) -> p h w", w=W),
                    x2e[b, :, h0 : h0 + hc, :],
                )

                atlases = {
                    (k, hf): atlas_pool.tile(
                        [64, hc * GW],
                        BF16,
                        tag=f"at{k}{hf}",
                        name=f"at{k}{hf}_{b}_{ci}",
                    )
                    for k, hf in SLICES
                }

                ncop = 0
                for g0 in range(0, hc, 4):
                    ng = min(4, hc - g0)
                    ps = [
                        psum_g.tile(
                            [wb, ng * GW],
                            F32,
                            tag=f"g{k}",
                            name=f"g{k}_{b}_{ci}_{g0}",
                        )
                        for k, (w0, wb) in enumerate(WBLOCKS)
                    ]
                    for q in range(ng):
                        hh = g0 + q
                        o1 = hh * W
                        o2 = hh * W
                        for kblk, (w0, wb) in enumerate(WBLOCKS):
                            for half in range(wb // 64):
                                ws = w0 + 64 * half  # global column of row 0
                                # window in unpadded x2 coords: [ws-8, ws+96)
                                lo = max(ws - PAD_L, 0)
                                hi = min(ws + GW - PAD_L, W)
                                joff = lo - (ws - PAD_L)  # left clip
                                nc.tensor.matmul(
                                    ps[kblk][
                                        half * 64 : half * 64 + 64,
                                        q * GW + joff : q * GW + joff + (hi - lo),
                                    ],
                                    x1b[:, o1 + ws : o1 + ws + 64],
                                    x2b[:, o2 + lo : o2 + hi],
                                    start=True,
                                    stop=True,
                                    tile_position=(0, 64 * half),
                                )
                    for kblk, hf in SLICES:
                        src = ps[kblk][hf * 64 : (hf + 1) * 64, :]
                        dst = atlases[(kblk, hf)][:, g0 * GW : (g0 + ng) * GW]
                        if ncop % 2 == 0:
                            nc.scalar.copy(dst, src)
                        else:
                            nc.vector.tensor_copy(dst, src)
                        ncop += 1

                # zero the pad corners the narrowed edge matmuls skipped:
                #  (i<8, j<8) of slice (0,0) and (i>=32, j>=72) of slice (2,0)
                a00 = atlases[(0, 0)][:].rearrange("p (h g) -> p h g", g=GW)
                nc.gpsimd.memset(a00[0:PAD_L, :, 0:PAD_L], 0.0)
                a20 = atlases[(2, 0)][:].rearrange("p (h g) -> p h g", g=GW)
                nc.gpsimd.memset(a20[PAD_R : 64, :, GW - PAD_R : GW], 0.0)

                sbig = [
                    sbig_pool.tile(
                        [128, hc * D], BF16, tag="sbig0", name=f"sbig0_{b}_{ci}"
                    ),
                    sbig_pool.tile(
                        [128, hc * D], BF16, tag="sbig1", name=f"sbig1_{b}_{ci}"
                    ),
                    sbig_pool.tile(
                        [64, hc * D], BF16, tag="sbig2", name=f"sbig2_{b}_{ci}"
                    ),
                ]
                for si, (kblk, hf) in enumerate(SLICES):
                    scr = scratch_pool.tile(
                        [64, hc * GW],
                        BF16,
                        tag=f"scr{kblk}{hf}",
                        name=f"scr{kblk}{hf}_{b}_{ci}",
                    )
                    store_eng = nc.sync if si % 2 == 0 else nc.scalar
                    store_eng.dma_start(scr[:], atlases[(kblk, hf)][:])

                    scr_ap = scr[:]
                    diag = AP(
                        tensor=scr_ap.tensor,
                        offset=scr_ap.offset,
                        ap=[[hc * GW + 1, 64], [GW, hc], [1, D]],
                    )
                    dstp = sbig[kblk][hf * 64 : (hf + 1) * 64, :].rearrange(
                        "p (h d) -> p h d", d=D
                    )
                    load_eng = nc.scalar if si % 2 == 0 else nc.sync
                    load_eng.dma_start(dstp, diag)
                return sbig

            def emit_back(b, ci, sbig):
                """Transposes, assembly copies, output store."""
                h0 = ci * hc
                abatch = asm_pool.tile(
                    [D, hc * W], F32, tag="abatch", name=f"abatch_{b}_{ci}"
                )
                for hh in range(hc):
                    t_ps = psum_t.tile(
                        [D, W], BF16, tag="t_ps", name=f"t_ps_{b}_{ci}_{hh}"
                    )
                    for kblk, (w0, wb) in enumerate(WBLOCKS):
                        nc.tensor.matmul(
                            t_ps[:, w0 : w0 + wb],
                            sbig[kblk][0:wb, hh * D : (hh + 1) * D],
                            identity[0:wb, 0:wb],
                            start=True,
                            stop=True,
                            is_transpose=True,
                        )
                    dst = abatch[:, hh * W : (hh + 1) * W]
                    if hh % 2 == 0:
                        nc.vector.tensor_copy(dst, t_ps[:])
                    else:
                        nc.scalar.copy(dst, t_ps[:])

                # SWDGE spreads descriptors across all 16 SDMA engines;
                # HWDGE pins this 41-partition store onto a single engine.
                nc.gpsimd.dma_start(
                    oute[b, :, h0 : h0 + hc, :],
                    abatch[:].rearrange("d (h w) -> d h w", w=W),
                )

            prev = None
            for b in range(b_dim):
                for ci in range(nchunks):
                    sbig = emit_front(b, ci)
                    if prev is not None:
                        emit_back(*prev)
                    prev = (b, ci, sbig)
            emit_back(*prev)

    nc.finalize()
    return nc


_compiled = {}


def _get_kernel(b_dim, hs):
    key = (b_dim, hs)
    if key not in _compiled:
        _compiled[key] = build_kernel(b_dim, hs)
    return _compiled[key]


def kernel(x_1: np.ndarray, x_2: np.ndarray) -> np.ndarray:
    assert x_1.shape == (B, C, H, W) and x_2.shape == (B, C, H, W)
    x_1 = np.ascontiguousarray(x_1, dtype=np.float32)
    x_2 = np.ascontiguousarray(x_2, dtype=np.float32)
    nc = _get_kernel(B, HS)
    in_maps = [
        {
            "x1": np.ascontiguousarray(x_1[:, :, i * HS : (i + 1) * HS, :]),
            "x2": np.ascontiguousarray(x_2[:, :, i * HS : (i + 1) * HS, :]),
        }
        for i in range(N_CORES)
    ]
    res = run_bass_kernel_spmd(nc, in_maps, core_ids=list(range(N_CORES))).results
    out = np.concatenate([res[i]["out"] for i in range(N_CORES)], axis=2)
    return out
